# revision 7
# baseline (speedup 1.0000x reference)
"""AverageDistanceLoss (ADD / ADD-S with margin) on 8 Trainium2 NeuronCores.

Math (reference semantics):
  per ROI b with label l>0, R1=quat_to_rotmat(pred), R2=quat_to_rotmat(target),
  pts = points[l] (a_q columns), x1 = R1 a, x2 = R2 a:
    non-sym: d[p] = ||R1 a_p - R2 a_p||^2
    sym:     d[p] = min_q (||R1 a_p||^2 + ||R2 a_q||^2 - 2 a_p^T (R1^T R2) a_q)
  loss = sum_b,p max(0.5 d - 0.01, 0) / (B*P)

Device strategy (per core, SPMD over 8 cores):
  - host shards valid ROIs: symmetric ROIs and non-symmetric ROIs round-robin
    across cores, padded with identity-quaternion slots (contribute exactly 0).
  - host sends per-slot tables tabs[r] = points[label].T  [3,1024] and
    quats q[r] = (q1, q2).
  - device computes R1, R2 (batched over slots on partitions), G = -2 R1^T R2,
    transposes them into [3, 4*slot] master tiles, then per symmetric slot:
      z  = R2 A            (PE, f32r)        -> ||R2 a_q||^2 via ACT Square
      y  = G A             (PE, f32r)
      M[p,q] = A^T y + 1^T z^2   (two accumulating K=3 f32r matmuls, 8 x 1024)
      minq = reduce_min over q   (DVE, from PSUM)
      n1[p] = ||R1 a_p||^2 partition-layout (8 tiny matmuls + Square + reduce)
      hinge = Relu(0.5*(n1+minq) - 0.01)
    per non-symmetric slot: w1, w2 partition-layout rotations, then
      hinge = Relu(0.5*sum((w1-w2)^2) - 0.01)
  - per-core scalar = partition-sum of all hinges (ones matmul); host sums the
    8 scalars and divides by B*P.
"""
import sys
import types
import numpy as np
from contextlib import ExitStack

import concourse.tile as tile
from concourse import bacc, mybir
from concourse.bass_utils import run_bass_kernel_spmd

F32 = mybir.dt.float32
F32R = mybir.dt.float32r
AX = mybir.AxisListType
ALU = mybir.AluOpType
ACTF = mybir.ActivationFunctionType

N_CORES = 8
B, C, P = 128, 22, 1024
MARGIN = 0.01

# ---------------------------------------------------------------------------
# Optional NTFF profiling support (used by test.py via BASS_TRACE=1).
# The agent image lacks antenv.axon_hooks; provide it so trace=True works.
try:
    import antenv.axon_hooks  # noqa: F401
except ImportError:
    _hooks = types.ModuleType("antenv.axon_hooks")
    _hook_store = [None]
    _hooks.set_axon_ntff_profile_hook = lambda h: _hook_store.__setitem__(0, h)
    _hooks.get_axon_ntff_profile_hook = lambda: _hook_store[0]
    sys.modules["antenv.axon_hooks"] = _hooks

    def _try_install_ntff_hook():
        try:
            from trn_agent_boot.trn_boot import _ntff_profile_via_ctypes
            h = _ntff_profile_via_ctypes("/opt/axon/libaxon_pjrt.so")
            if h is not None:
                _hooks.set_axon_ntff_profile_hook(h)
        except Exception:
            pass

    _try_install_ntff_hook()

last_results = None  # BassKernelResults of the most recent run (for test.py)

_program_cache = {}


def _emit_quat_entries(nc, Re, Pt):
    """Re[:, 3j+i] = R[i, j] from pre-2x-scaled products Pt
    (cols: uu,vv,ww,uv,uw,vw,su,sv,sw, all already scaled by 2)."""
    v = nc.vector
    # diag pre-sums: vv+ww, uu+ww, uu+vv
    D = Re[:, 12:15]  # scratch columns (junk area of the 32-wide tile)
    v.tensor_add(D[:, 0:1], Pt[:, 1:2], Pt[:, 2:3])
    v.tensor_add(D[:, 1:2], Pt[:, 0:1], Pt[:, 2:3])
    v.tensor_add(D[:, 2:3], Pt[:, 0:1], Pt[:, 1:2])
    for e, c in ((0, 0), (4, 1), (8, 2)):  # 1 - (sum)
        nc.scalar.activation(Re[:, e:e + 1], D[:, c:c + 1], ACTF.Copy,
                             bias=1.0, scale=-1.0)
    # off-diag: e = 3j+i holds R[i,j]
    v.tensor_add(Re[:, 1:2], Pt[:, 3:4], Pt[:, 8:9])   # R[1,0] = uv+sw
    v.tensor_sub(Re[:, 3:4], Pt[:, 3:4], Pt[:, 8:9])   # R[0,1] = uv-sw
    v.tensor_sub(Re[:, 2:3], Pt[:, 4:5], Pt[:, 7:8])   # R[2,0] = uw-sv
    v.tensor_add(Re[:, 6:7], Pt[:, 4:5], Pt[:, 7:8])   # R[0,2] = uw+sv
    v.tensor_add(Re[:, 5:6], Pt[:, 5:6], Pt[:, 6:7])   # R[2,1] = vw+su
    v.tensor_sub(Re[:, 7:8], Pt[:, 5:6], Pt[:, 6:7])   # R[1,2] = vw-su


def build_program(S, NS):
    """Build the SPMD program for S symmetric + NS non-symmetric slots/core."""
    R = S + NS
    assert 1 <= R <= 32
    nc = bacc.Bacc("TRN2", target_bir_lowering=False, debug=False,
                   num_devices=N_CORES)
    q_in = nc.declare_dram_parameter("q", [32, 8], F32, isOutput=False)
    tabs_in = nc.declare_dram_parameter("tabs", [R, 4, P], F32R, isOutput=False)
    out_d = nc.declare_dram_parameter("out", [1], F32, isOutput=True)

    with tile.TileContext(nc) as tc:
        with ExitStack() as ctx:
            sing = ctx.enter_context(tc.tile_pool(name="sing", bufs=1))
            work = ctx.enter_context(tc.tile_pool(name="work", bufs=2))
            y3p = ctx.enter_context(tc.tile_pool(name="y3p", bufs=2))
            z2p = ctx.enter_context(tc.tile_pool(name="z2p", bufs=2))
            pwp = ctx.enter_context(tc.tile_pool(name="pwp", bufs=2,
                                                 space="PSUM"))
            smp = ctx.enter_context(tc.tile_pool(name="smp", bufs=2,
                                                 space="PSUM"))

            # ---- per-slot point tables ------------------------------------
            A = []
            for r in range(R):
                a = sing.tile([4, P], F32R, tag=f"A{r}")
                nc.gpsimd.dma_start(a[:], tabs_in[r])
                A.append(a)

            # ---- quaternions -> R1, R2, G = -2 R1^T R2 --------------------
            Q = sing.tile([32, 8], F32, tag="Q")
            nc.gpsimd.dma_start(Q[:], q_in[:])
            Q2 = sing.tile([32, 8], F32, tag="Q2")
            nc.scalar.activation(Q2[:], Q[:], ACTF.Copy, bias=0.0,
                                 scale=float(np.sqrt(2.0)))
            R1e = sing.tile([32, 32], F32, tag="R1e")
            R2e = sing.tile([32, 32], F32, tag="R2e")
            Ge = sing.tile([32, 32], F32, tag="Ge")
            for Re, off in ((R1e, 0), (R2e, 4)):
                Pt = Re[:, 16:25]  # scratch: products live in cols 16..24
                s_, u_, v_, w_ = (Q2[:, off + k:off + k + 1] for k in range(4))
                uvw = Q2[:, off + 1:off + 4]
                nc.vector.tensor_mul(Re[:, 16:19], uvw, uvw)      # uu,vv,ww
                nc.vector.tensor_mul(Re[:, 19:20], u_, v_)        # uv
                nc.vector.tensor_mul(Re[:, 20:21], u_, w_)        # uw
                nc.vector.tensor_mul(Re[:, 21:22], v_, w_)        # vw
                nc.vector.tensor_mul(Re[:, 22:23], s_, u_)        # su
                nc.vector.tensor_mul(Re[:, 23:24], s_, v_)        # sv
                nc.vector.tensor_mul(Re[:, 24:25], s_, w_)        # sw
                _emit_quat_entries(nc, Re, Pt)
            # G[i,j] = sum_k R1[k,i] R2[k,j]; entry col 3j+i; then scale -2
            for j in range(3):
                nc.vector.tensor_scalar_mul(
                    Ge[:, 3 * j:3 * j + 3], R1e[:, 0:9:3],
                    R2e[:, 3 * j:3 * j + 1])
                for k in (1, 2):
                    nc.vector.scalar_tensor_tensor(
                        Ge[:, 3 * j:3 * j + 3], R1e[:, k:9:3],
                        R2e[:, 3 * j + k:3 * j + k + 1],
                        Ge[:, 3 * j:3 * j + 3],
                        op0=ALU.mult, op1=ALU.add)
            nc.scalar.activation(Ge[:, 0:9], Ge[:, 0:9], ACTF.Copy,
                                 bias=0.0, scale=-2.0)

            # ---- transpose into [3, 4R] master tiles ----------------------
            T1 = sing.tile([32, 32], F32, tag="T1")
            T2 = sing.tile([32, 32], F32, tag="T2")
            TG = sing.tile([32, 32], F32, tag="TG")
            nc.vector.transpose(T1[:], R1e[:])
            nc.vector.transpose(T2[:], R2e[:])
            nc.vector.transpose(TG[:], Ge[:])
            RT1 = sing.tile([3, 4 * R], F32R, tag="RT1")
            RT2 = sing.tile([3, 4 * R], F32R, tag="RT2")
            RTD = sing.tile([3, 4 * R], F32R, tag="RTD")  # R1 - R2 (non-sym)
            # LTG: lhsT of the y-matmul. Per slot r: LTG[j, 4r+i] = G_r[i, j]
            # for i<3, col 4r+3 = 0. onesLT: [0,0,0 | 1] columns (z^2 sum).
            LTG = sing.tile([3, 4 * R], F32R, tag="LTG")
            zr3 = sing.tile([3, 4 * R], F32, tag="zr3")
            nc.vector.memset(zr3[:], 0.0)
            nc.scalar.copy(LTG[:], zr3[:])
            onesLT = sing.tile([3, 4], F32R, tag="onesLT")
            zf4 = sing.tile([3, 4], F32, tag="zf4")
            nc.vector.memset(zf4[:, 0:3], 0.0)
            nc.vector.memset(zf4[:, 3:4], 1.0)
            nc.scalar.copy(onesLT[:], zf4[:])
            for RT, T in ((RT1, T1), (RT2, T2), (LTG, TG)):
                for j in range(3):
                    for i in range(3):
                        # RT[j, 4r+i] = T[3j+i, r]
                        dst = RT[j:j + 1, i:4 * R:4]
                        src = T[3 * j + i:3 * j + i + 1, 0:R].bitcast(F32R)
                        nc.gpsimd.dma_start(dst, src)
            nc.vector.tensor_sub(RTD[:], RT1[:], RT2[:])

            # ---- constants ------------------------------------------------
            onescol = sing.tile([128, 1], F32, tag="onescol")
            nc.vector.memset(onescol[:], 1.0)
            biasc = sing.tile([128, 1], F32, tag="biasc")
            nc.vector.memset(biasc[:], -MARGIN)
            H = sing.tile([128, 8 * R], F32, tag="H")

            # ---- slot bodies (emitted software-pipelined) -----------------
            def emit_sym_prep(r):
                a = A[r]
                z_ps = smp.tile([3, P], F32, tag="zy")
                for n in range(2):
                    nsl = slice(512 * n, 512 * (n + 1))
                    nc.tensor.matmul(z_ps[:, nsl], RT2[0:3, 4 * r:4 * r + 3],
                                     a[0:3, nsl], start=True, stop=True)
                z2s = z2p.tile([3, P], F32R, tag="z2s")
                nc.scalar.square(z2s[:], z_ps[:])
                # Y4[0:3] = G A (from table); Y4[3] = sum_j z^2 (accumulate)
                y_ps = smp.tile([4, P], F32, tag="zy")
                for n in range(2):
                    nsl = slice(512 * n, 512 * (n + 1))
                    nc.tensor.matmul(y_ps[:, nsl], LTG[0:3, 4 * r:4 * r + 4],
                                     a[0:3, nsl], start=True, stop=False)
                    nc.tensor.matmul(y_ps[:, nsl], onesLT[:],
                                     z2s[0:3, nsl], start=False, stop=True)
                Y4 = y3p.tile([4, P], F32R, tag="Y4")
                nc.scalar.copy(Y4[:], y_ps[:])
                # n1[p] = ||R1 a_p||^2, partition layout [128, 8]
                w_ps = smp.tile([128, 32], F32, tag="zy")
                for t in range(8):
                    nc.tensor.matmul(w_ps[:, 4 * t:4 * t + 4],
                                     a[0:3, 128 * t:128 * (t + 1)],
                                     RT1[0:3, 4 * r:4 * r + 4],
                                     start=True, stop=True)
                w1sq = work.tile([128, 32], F32, tag="wsq")
                nc.scalar.square(w1sq[:], w_ps[:])
                n1pt = work.tile([128, 8], F32, tag="n1pt")
                nc.vector.tensor_reduce(
                    n1pt[:],
                    w1sq[:].rearrange("p (t i) -> p t i", i=4)[:, :, 0:3],
                    axis=AX.X, op=ALU.add)
                return Y4, n1pt

            def emit_sym_tail(r, Y4, n1pt):
                a = A[r]
                mincols = work.tile([128, 8], F32, tag="mincols")
                for t in range(8):
                    pw = pwp.tile([128, P], F32)
                    lA = a[0:4, 128 * t:128 * (t + 1)]
                    for n in range(2):
                        nsl = slice(512 * n, 512 * (n + 1))
                        nc.tensor.matmul(pw[:, nsl], lA, Y4[:, nsl],
                                         start=True, stop=True)
                    nc.vector.tensor_reduce(mincols[:, t:t + 1], pw[:],
                                            axis=AX.X, op=ALU.min)
                d = work.tile([128, 8], F32, tag="dtile")
                nc.vector.tensor_add(d[:], n1pt[:], mincols[:])
                nc.scalar.activation(H[:, 8 * r:8 * r + 8], d[:], ACTF.Relu,
                                     bias=biasc[:], scale=0.5)

            def emit_ns(r):
                a = A[r]
                wd = smp.tile([128, 32], F32, tag="zy")
                for t in range(8):
                    nc.tensor.matmul(wd[:, 4 * t:4 * t + 4],
                                     a[0:3, 128 * t:128 * (t + 1)],
                                     RTD[0:3, 4 * r:4 * r + 4],
                                     start=True, stop=True)
                dsq = work.tile([128, 32], F32, tag="nssq")
                nc.scalar.square(dsq[:], wd[:])
                dts = work.tile([128, 8], F32, tag="nsd")
                nc.vector.tensor_reduce(
                    dts[:],
                    dsq[:].rearrange("p (t i) -> p t i", i=4)[:, :, 0:3],
                    axis=AX.X, op=ALU.add)
                nc.scalar.activation(H[:, 8 * r:8 * r + 8], dts[:], ACTF.Relu,
                                     bias=biasc[:], scale=0.5)

            # pipeline: prep slot r+1 is emitted before the heavy tail of
            # slot r; non-symmetric slots are woven between sym tails.
            ns_list = list(range(S, R))
            ns_pos = 0
            stride = max(1, (S + len(ns_list)) // max(1, len(ns_list))) \
                if ns_list else 0
            pend = None
            if S > 0:
                pend = emit_sym_prep(0)
            for i in range(S):
                nxt = emit_sym_prep(i + 1) if i + 1 < S else None
                emit_sym_tail(i, *pend)
                pend = nxt
                if ns_list and ns_pos < len(ns_list) and stride and \
                        i % stride == stride - 1:
                    emit_ns(ns_list[ns_pos])
                    ns_pos += 1
            for k in range(ns_pos, len(ns_list)):
                emit_ns(ns_list[k])

            # ---- final reduction ------------------------------------------
            colsum = sing.tile([128, 1], F32, tag="colsum")
            nc.vector.tensor_reduce(colsum[:], H[:], axis=AX.X, op=ALU.add)
            ps = smp.tile([1, 1], F32, tag="zy")
            nc.tensor.matmul(ps[:], colsum[:], onescol[:], start=True,
                             stop=True)
            outs = sing.tile([1, 1], F32, tag="outs")
            nc.scalar.copy(outs[:], ps[:])
            nc.gpsimd.dma_start(out_d[:], outs[0, :])
    nc.compile()
    return nc


def _quat_ident():
    return np.array([1.0, 0, 0, 0, 1.0, 0, 0, 0], dtype=np.float32)


def kernel(poses_pred, poses_target, poses_labels, points, symmetry):
    global last_results
    poses_pred = np.asarray(poses_pred, dtype=np.float32)
    poses_target = np.asarray(poses_target, dtype=np.float32)
    poses_labels = np.asarray(poses_labels)
    points = np.asarray(points, dtype=np.float32)
    symmetry = np.asarray(symmetry)

    valid = poses_labels > 0
    is_sym = (symmetry[poses_labels] > 0) & valid
    is_ns = (~(symmetry[poses_labels] > 0)) & valid
    sym_idx = np.nonzero(is_sym)[0]
    ns_idx = np.nonzero(is_ns)[0]

    if len(sym_idx) == 0 and len(ns_idx) == 0:
        return np.float32(0.0)

    S = int(np.ceil(len(sym_idx) / N_CORES))
    NS = int(np.ceil(len(ns_idx) / N_CORES))
    R = S + NS

    key = (S, NS)
    if key not in _program_cache:
        _program_cache[key] = build_program(S, NS)
    nc = _program_cache[key]

    ptsT = np.concatenate([points.transpose(0, 2, 1),
                           np.ones((points.shape[0], 1, P), np.float32)],
                          axis=1)  # [C, 4, P]: rows 0-2 = pts^T, row 3 = ones
    ptsT = np.ascontiguousarray(ptsT)
    in_maps = []
    for k in range(N_CORES):
        q = np.zeros((32, 8), dtype=np.float32)
        tabs = np.empty((R, 4, P), dtype=np.float32)
        my_sym = sym_idx[k::N_CORES]
        my_ns = ns_idx[k::N_CORES]
        for r in range(S):
            if r < len(my_sym):
                b = my_sym[r]
                q[r, 0:4] = poses_pred[b, poses_labels[b]]
                q[r, 4:8] = poses_target[b, poses_labels[b]]
                tabs[r] = ptsT[poses_labels[b]]
            else:
                q[r] = _quat_ident()
                tabs[r] = ptsT[0]
        for i in range(NS):
            r = S + i
            if i < len(my_ns):
                b = my_ns[i]
                q[r, 0:4] = poses_pred[b, poses_labels[b]]
                q[r, 4:8] = poses_target[b, poses_labels[b]]
                tabs[r] = ptsT[poses_labels[b]]
            else:
                q[r] = _quat_ident()
                tabs[r] = ptsT[0]
        in_maps.append({"q": q, "tabs": tabs})

    res = run_bass_kernel_spmd(nc, in_maps, list(range(N_CORES)))
    last_results = res
    total = float(sum(float(res.results[k]["out"][0]) for k in range(N_CORES)))
    return np.float32(total / (B * P))
